# revision 2
# baseline (speedup 1.0000x reference)
"""Trainium2 Bass kernel for the entropy-regularized knapsack CVX loss.

Math: with e = x / (||x||_2 * TAU), the per-row solution of
    max e@z + EPS*sum(entr(z))  s.t. 0<=z<=1, sum z = K
is p_i = min(1, exp((e_i - nu)/EPS - 1)) with nu s.t. sum_i p_i = K.
Since |e_i| <= 1 (Cauchy-Schwarz) and n = 8192 >> K*e^2, the min(1,.)
clamp is never active at the optimum, so p = K * softmax(e) and
loss = mean(-log(K*exp(e_y)/s + 1e-8)) with s = sum_j exp(e_j).

Key reduction: ||e||_2 = 1/TAU = 1, so the 2nd-order Taylor expansion of
s around 0 is UNCONDITIONALLY accurate:
    s = sum exp(e_j) = N + sum e_j + 0.5*sum e_j^2 + R,
    |R| <= e/6 * (sum e_j^2)^{3/2} = e/6 ~ 0.45 abs  (vs s ~ N = 8192)
i.e. rel err <= 5.6e-5 for ANY row.  sum e_j^2 = 1 exactly, and the
linear term |sum e_j| <= sqrt(N) ~ 90 worst-case (~1 for real data)
contributes <= 1.1% worst-case / ~1e-4 typical to s.  Dropping it keeps
the loss within ~2e-5 relative (validated numerically against the
reference on the actual inputs; tolerance is 2e-2).

So the DEVICE only needs the per-row sum of squares S2 = sum_j x_ij^2
(which gives both the norm and the quadratic term: c = 1/sqrt(S2),
c^2*S2 = 1).  Host does the O(B) rest: gather x[r, y[r]], p_y =
K*exp(x_y*c)/(N + 0.5), loss = mean(-log(p_y + 1e-8)).

Device kernel per 128-row tile (data-parallel over 8 cores, 8 tiles
each): DMA the tile in fp8-e4m3 (or bf16), then split columns between
the Scalar engine (Square activation with fused free-dim accumulate,
1 elem/cycle @ 1.2 GHz, dtype-independent) and the Vector engine
(scalar_tensor_tensor x*x with fused accumulate, 1 elem/cycle @ 0.96
GHz at fp8), so the square-reduce of every element runs at the
combined ACT+DVE rate and the kernel tracks the fp8 DMA roofline.
fp8 quantization only perturbs the NORM (not e_y, which the host
computes from full-precision x): S2 rel err ~0.1% -> loss rel err
~1e-5 (validated).  Exact f64 fallback for any row with nonfinite or
nonpositive S2 (never for real data).
"""

import numpy as np

_BATCH = 8192
_N = 8192
_NCORES = 8
_RPC = _BATCH // _NCORES  # rows per core
_P = 128
_TILES = _RPC // _P  # row-tiles per core
_K = 5.0
_TAU = 1.0
_EPS = 1.0

_NC_CACHE = {}
VARIANT = "sq8"

# fp8 column split: ACT gets [0:A), DVE gets [A:N).
# Balance (A+352)/1.2GHz == (N-A)/0.96GHz  ->  A ~ 4416.
_ACT_COLS_8 = 4416
# with GPSIMD assist: g=T/2 @2.0ns/col, DVE also re-reduces gpsimd's
# x^2 scratch at 4x bf16.  T~3383ns: g~1692, a~3710, d~2790.
_GP_COLS = 1664
_ACT_COLS_GP = 3712


def _dtype_of(variant):
    return "f8" if variant.endswith("8") or variant.endswith("8gp") else "bf16"


def _build_bass(repeat=1, variant="sq8"):
    import concourse.bacc as bacc
    import concourse.mybir as mybir
    import concourse.tile as tile

    nc = bacc.Bacc(
        "TRN2", target_bir_lowering=False, debug=False, num_devices=_NCORES
    )
    f32 = mybir.dt.float32
    bf16 = mybir.dt.bfloat16
    f8 = mybir.dt.float8e4
    AF = mybir.ActivationFunctionType
    ALU = mybir.AluOpType

    is8 = variant in ("sq8", "sq8gp", "dma8")
    x_dt = f8 if is8 else bf16

    x = nc.dram_tensor("x", [_RPC, _N], x_dt, kind="ExternalInput")
    stats = nc.dram_tensor(
        "stats", [_P, 2 * _TILES], f32, kind="ExternalOutput"
    )

    with tile.TileContext(nc) as tc:
        with (
            tc.tile_pool(name="xp", bufs=4) as xp,
            tc.tile_pool(name="sp", bufs=2) as sp,
            tc.tile_pool(name="singles", bufs=1) as singles,
        ):
            stats_sb = singles.tile([_P, 2 * _TILES], f32)
            nc.vector.memset(stats_sb, 0.0)

            def tile_body(t):
                x_tile = xp.tile([_P, _N], x_dt, tag="x", name=f"x_{t}")
                nc.sync.dma_start(out=x_tile, in_=x[t * _P : (t + 1) * _P, :])
                if variant in ("dma8", "dma16"):
                    return
                sA = stats_sb[:, 2 * t : 2 * t + 1]
                sD = stats_sb[:, 2 * t + 1 : 2 * t + 2]

                if variant in ("sq8", "sq8gp"):
                    a_cols = _ACT_COLS_GP if variant == "sq8gp" else _ACT_COLS_8
                    g_cols = _GP_COLS if variant == "sq8gp" else 0
                    # ACT: Square with fused accumulate over cols [0:a)
                    scrA = sp.tile([_P, a_cols], f8, tag="scrA", name=f"sa_{t}")
                    nc.scalar.activation(
                        scrA, x_tile[:, :a_cols], AF.Square, accum_out=sA
                    )
                    # DVE: (x*1)*x with fused accumulate over [a:N-g)
                    d0, d1 = a_cols, _N - g_cols
                    scrD = sp.tile(
                        [_P, d1 - d0], f8, tag="scrD", name=f"sd_{t}"
                    )
                    nc.vector.scalar_tensor_tensor(
                        out=scrD,
                        in0=x_tile[:, d0:d1],
                        scalar=1.0,
                        in1=x_tile[:, d0:d1],
                        op0=ALU.mult,
                        op1=ALU.mult,
                        accum_out=sD,
                    )
                    if g_cols:
                        # GPSIMD squares the tail into bf16 scratch; DVE
                        # re-reduces it at 4x and chains into sD later on
                        # host (extra stats col pair not needed: reuse sA
                        # accum via tensor_scalar accum into scratch col).
                        scrG = sp.tile(
                            [_P, g_cols], bf16, tag="scrG", name=f"sg_{t}"
                        )
                        nc.gpsimd.tensor_mul(
                            out=scrG, in0=x_tile[:, d1:], in1=x_tile[:, d1:]
                        )
                        scrG2 = sp.tile(
                            [_P, g_cols], bf16, tag="scrG2", name=f"sg2_{t}"
                        )
                        gacc = sp.tile(
                            [_P, 1], f32, tag="gacc", name=f"ga_{t}"
                        )
                        nc.vector.tensor_scalar(
                            out=scrG2,
                            in0=scrG,
                            scalar1=1.0,
                            scalar2=None,
                            op0=ALU.mult,
                            accum_out=gacc,
                        )
                        # fold into sD on DVE ([P,1] add, negligible)
                        nc.vector.tensor_tensor(
                            out=sD, in0=sD, in1=gacc, op=ALU.add
                        )
                elif variant == "sq16":
                    # tile-granular split: even tiles on ACT, odd on DVE
                    if t % 2 == 0:
                        scrA = sp.tile([_P, _N], bf16, tag="scrA", name=f"sa_{t}")
                        nc.scalar.activation(
                            scrA, x_tile, AF.Square, accum_out=sA
                        )
                    else:
                        sq = sp.tile([_P, _N], bf16, tag="scrD", name=f"sd_{t}")
                        nc.vector.tensor_mul(out=sq, in0=x_tile, in1=x_tile)
                        sq2 = sp.tile([_P, _N], bf16, tag="scrD2", name=f"se_{t}")
                        nc.vector.tensor_scalar(
                            out=sq2,
                            in0=sq,
                            scalar1=1.0,
                            scalar2=None,
                            op0=ALU.mult,
                            accum_out=sD,
                        )
                else:
                    raise ValueError(variant)

            def body():
                for t in range(_TILES):
                    tile_body(t)

            if repeat == 1:
                body()
            else:
                with tc.For_i(0, repeat, 1):
                    body()
            nc.sync.dma_start(out=stats[:, :], in_=stats_sb)
    nc.finalize()
    return nc


def _get_nc(repeat=1, variant=None):
    if variant is None:
        variant = VARIANT
    key = (repeat, variant)
    if key not in _NC_CACHE:
        _NC_CACHE[key] = _build_bass(repeat, variant)
    return _NC_CACHE[key]


def _cast_for(variant, arr):
    import ml_dtypes

    if variant in ("sq8", "sq8gp", "dma8"):
        return arr.astype(ml_dtypes.float8_e4m3)
    return arr.astype(ml_dtypes.bfloat16)


def _exact_p_y(xrows, yrows):
    """f64 exact solve of the knapsack dual for fallback rows."""
    xr = np.asarray(xrows, dtype=np.float64)
    if xr.ndim == 1:
        xr = xr[None, :]
    n = xr.shape[1]
    norm = np.maximum(np.sqrt((xr * xr).sum(1, keepdims=True)), 1e-12)
    e = xr / norm / _TAU
    lo = e.min(1) - _EPS
    hi = e.max(1) + _EPS * np.log(float(n))
    for _ in range(200):
        mid = 0.5 * (lo + hi)
        f = np.minimum(1.0, np.exp((e - mid[:, None]) / _EPS - 1.0)).sum(1)
        big = f > _K
        lo = np.where(big, mid, lo)
        hi = np.where(big, hi, mid)
    nu = 0.5 * (lo + hi)
    e_y = e[np.arange(e.shape[0]), yrows]
    return np.minimum(1.0, np.exp((e_y - nu) / _EPS - 1.0))


def kernel(x, y):
    from concourse.bass_utils import run_bass_kernel_spmd

    x = np.asarray(x, dtype=np.float32)
    y = np.asarray(y).astype(np.int64)
    assert x.shape == (_BATCH, _N)

    nc = _get_nc()
    in_maps = [
        {
            "x": np.ascontiguousarray(
                _cast_for(VARIANT, x[i * _RPC : (i + 1) * _RPC])
            )
        }
        for i in range(_NCORES)
    ]
    res = run_bass_kernel_spmd(nc, in_maps, core_ids=list(range(_NCORES)))

    parts = []
    for r in res.results:
        st = r["stats"].astype(np.float64)  # [P, 2*TILES]
        # S2 for row (t*128+p) of this core = st[p, 2t] + st[p, 2t+1]
        s2 = (st[:, 0::2] + st[:, 1::2]).T.reshape(-1)  # [RPC]
        parts.append(s2)
    S2 = np.concatenate(parts)  # [BATCH]

    rows = np.arange(_BATCH)
    x_y = x[rows, y].astype(np.float64)
    with np.errstate(all="ignore"):
        c = 1.0 / (np.sqrt(S2) * _TAU)
        e_y = x_y * c
        s = float(_N) + 0.5  # N + c*S1(dropped) + 0.5*c^2*S2 (== 0.5)
        p_y = np.minimum(1.0, _K * np.exp(e_y) / s)
        bad = ~(np.isfinite(p_y) & (S2 > 0))
    if bad.any():
        p_y[bad] = _exact_p_y(x[bad], y[bad])
    loss = np.mean(-np.log(p_y + 1e-8))
    return np.array(loss, dtype=np.float32)
